# revision 1
# baseline (speedup 1.0000x reference)
"""Trainium2 Bass kernel for nn_DMLoss_61942018343083 (Chamfer-style polygon
matching loss, retrieval_knn).

Sharding: data-parallel over batch B=32 across 8 NeuronCores (4 batches/core).
Each core computes three partial sums into a [128, 12] output tile; the host
combines them into the scalar loss.

Per batch (Np = Ng = 512, T = 10, Ngi = 5120 interp points):

pred2gt (argmin over 5120 interp points for each of 512 preds):
  * Approximate ranking key on the TensorEngine:
      key[p, g'] = 2*a_t*(px*gx[i] + py*gy[i]) + 2*b_t*(px*gxr[i] + py*gyr[i])
                   - (a_t^2*u[i] + 2*a_t*b_t*v[i] + b_t^2*w[i])
    with g' = t*512 + i (t-major), u=|gt[i]|^2, v=gt[i].gt[i-1], w=u[i-1].
    key is a monotone-decreasing proxy of the squared distance per row, so
    argmax(key) ~ argmin(d).  One K=7 matmul per (pred-chunk, t).
  * nc.vector.max / max_index give the top-8 candidates per pred.
  * Exact refine: gather 4 candidate coords from the interp table (built
    on-device with bit-exact reference rounding), recompute the 4 distances
    with the exact fp32 reference formula, pick the true min.  Empirically the
    true argmin always ranks <= 2 in the key (margin to rank 8 is >= 13.7 in
    squared-distance units vs key error <= ~0.5), so the result is bit-exact.

gt2pred (argmin over 512 preds for each of 512 gts):
  * Exact elementwise squared distances: replicate pred rows across
    partitions (DMA broadcast), ACT Square with per-partition bias, DVE add.
  * Negate -> max/max_index = exact argmin (first-index ties like jnp.argmin).
  * Gather winning pred_polys_ row, masked abs-diff partial sums.
"""

import os
import sys

for _p in ("/opt/trn_rl_repo", "/root/.axon_site/_ro/trn_rl_repo"):
    if os.path.isdir(_p) and _p not in sys.path:
        sys.path.insert(0, _p)

import numpy as np

import concourse.bass as bass
import concourse.bacc as bacc
import concourse.mybir as mybir
from concourse.bass import IndirectOffsetOnAxis
from concourse.bass_utils import run_bass_kernel_spmd
from concourse.tile import TileContext
from concourse.tile_rust import add_dep_helper

F32 = mybir.dt.float32
U32 = mybir.dt.uint32
AF = mybir.ActivationFunctionType
ALU = mybir.AluOpType
AX = mybir.AxisListType

B, NP, NG, T = 32, 512, 512, 10
NCORES = 8
BLOC = B // NCORES          # 4 batches per core
NGI = NG * T                # 5120 interpolated gt points
NCH = NP // 128             # 4 chunks of 128 preds (also 4 chunks of 128 gts)
KC = 4                      # candidates kept for the exact refine


def _coef_tables():
    """fp32-exact interpolation coefficients (match jnp.arange(T)/T)."""
    f = np.float32
    a = (np.arange(T, dtype=np.float32) / f(T)).astype(np.float32)       # t/10
    b = (f(1.0) - a).astype(np.float32)                                  # 1 - t/10
    coef = np.zeros((7, T), dtype=np.float32)
    coef[0] = (f(2.0) * a).astype(np.float32)
    coef[1] = coef[0]
    coef[2] = (f(2.0) * b).astype(np.float32)
    coef[3] = coef[2]
    coef[4] = (a * a).astype(np.float32)
    coef[5] = (f(2.0) * (a * b).astype(np.float32)).astype(np.float32)
    coef[6] = (b * b).astype(np.float32)
    ab = np.stack([a, b], axis=1).astype(np.float32)                     # [10, 2]
    return coef, ab


def build_nc():
    nc = bacc.Bacc()

    ini = nc.dram_tensor("ini_pred_poly", [BLOC, NP, 2], F32, kind="ExternalInput")
    pred2 = nc.dram_tensor("pred_polys_", [BLOC, NP, 2], F32, kind="ExternalInput")
    gt = nc.dram_tensor("gt_polys", [BLOC, NG, 2], F32, kind="ExternalInput")
    kmask = nc.dram_tensor("keyPointsMask", [BLOC, NG], F32, kind="ExternalInput")
    coef7 = nc.dram_tensor("coef7", [7, T], F32, kind="ExternalInput")
    abcol = nc.dram_tensor("abcol", [T, 2], F32, kind="ExternalInput")
    out = nc.dram_tensor("out", [128, 12], F32, kind="ExternalOutput")

    # per-batch gather tables (separate tensors -> AP offset 0 as required by
    # indirect_dma_start)
    itabs = [nc.dram_tensor(f"itab{b_}", [NGI, 2], F32) for b_ in range(BLOC)]
    ptabs = [nc.dram_tensor(f"ptab{b_}", [NP, 2], F32) for b_ in range(BLOC)]

    with TileContext(nc) as tc:
        with (
            tc.tile_pool(name="const", bufs=1) as cpool,
            tc.tile_pool(name="rows", bufs=1) as rows,
            tc.tile_pool(name="key", bufs=2) as keyp,
            tc.tile_pool(name="small", bufs=3) as small,
            tc.tile_pool(name="rhs", bufs=T + 1) as rhsp,
            tc.tile_pool(name="lhs", bufs=NCH + 2) as lhsp,
            tc.tile_pool(name="g2p", bufs=2) as g2p,
            tc.tile_pool(name="kps", bufs=3, space="PSUM") as kps,
            tc.tile_pool(name="repps", bufs=1, space="PSUM") as repps,
            tc.tile_pool(name="prep", bufs=2, space="PSUM") as prep,
        ):
            ones = cpool.tile([1, 128], F32)
            nc.vector.memset(ones[:], 1.0)
            coef_sb = cpool.tile([7, T], F32)
            nc.sync.dma_start(out=coef_sb[:], in_=coef7[:])
            ab_sb = cpool.tile([T, 2], F32)
            nc.sync.dma_start(out=ab_sb[:], in_=abcol[:])
            res = cpool.tile([128, 12], F32)

            for b_ in range(BLOC):
                # ---------- per-batch base rows ----------
                base7 = rows.tile([7, NG], F32)     # gx, gy, gxr, gyr, u, v, w
                flat = rows.tile([1, 2 * NG], F32)  # gt[b] flattened (x,y pairs)
                flatr = rows.tile([1, 2 * NG], F32)  # rolled by one point
                for c in range(2):
                    nc.sync.dma_start(out=base7[c:c + 1, :], in_=gt[b_:b_ + 1, :, c])
                    nc.sync.dma_start(out=base7[2 + c:3 + c, 0:1],
                                      in_=gt[b_:b_ + 1, NG - 1:NG, c])
                    nc.sync.dma_start(out=base7[2 + c:3 + c, 1:NG],
                                      in_=gt[b_:b_ + 1, 0:NG - 1, c])
                nc.sync.dma_start(out=flat[:], in_=gt[b_:b_ + 1, :, :])
                nc.sync.dma_start(out=flatr[0:1, 0:2], in_=gt[b_:b_ + 1, NG - 1:NG, :])
                nc.sync.dma_start(out=flatr[0:1, 2:2 * NG],
                                  in_=gt[b_:b_ + 1, 0:NG - 1, :])

                # u, v, w computed in partition-0 tiles (engine outputs must be
                # 32-aligned), then DMA'd into base7 partitions 4..6
                sq = rows.tile([1, 2 * NG], F32)
                nc.vector.tensor_tensor(out=sq[:], in0=flat[:], in1=flat[:],
                                        op=ALU.mult)
                sqv = sq.rearrange("p (i two) -> p i two", two=2)
                urow = rows.tile([1, NG], F32)
                nc.vector.tensor_tensor(out=urow[:], in0=sqv[:, :, 0],
                                        in1=sqv[:, :, 1], op=ALU.add)  # u
                pr = rows.tile([1, 2 * NG], F32)
                nc.vector.tensor_tensor(out=pr[:], in0=flat[:], in1=flatr[:],
                                        op=ALU.mult)
                prv = pr.rearrange("p (i two) -> p i two", two=2)
                vrow = rows.tile([1, NG], F32)
                nc.vector.tensor_tensor(out=vrow[:], in0=prv[:, :, 0],
                                        in1=prv[:, :, 1], op=ALU.add)  # v
                nc.sync.dma_start(out=base7[4:5, :], in_=urow[:])
                nc.sync.dma_start(out=base7[5:6, :], in_=vrow[:])
                # w = roll(u, 1)
                nc.sync.dma_start(out=base7[6:7, 1:NG], in_=urow[0:1, 0:NG - 1])
                nc.sync.dma_start(out=base7[6:7, 0:1], in_=urow[0:1, NG - 1:NG])

                # ---------- exact interp table (t-major), stored to DRAM ----------
                # replicate flat/flatr across 10 partitions via K=1 ones-matmul
                # (exact: single-term fp32 accumulate of 1*x), then scale by
                # a_t/b_t per partition (exact single rounding) and add.
                m1 = rows.tile([T, 2 * NG], F32)
                m2 = rows.tile([T, 2 * NG], F32)
                tab = rows.tile([T, 2 * NG], F32)
                for half in range(2):
                    hs = slice(NG * half, NG * (half + 1))
                    ps_f = repps.tile([T, NG], F32, tag="repps")
                    nc.tensor.matmul(ps_f[:], lhsT=ones[0:1, 0:T],
                                     rhs=flat[0:1, hs], start=True, stop=True)
                    nc.vector.tensor_scalar(out=m1[:, hs], in0=ps_f[:],
                                            scalar1=ab_sb[:, 0:1], scalar2=None,
                                            op0=ALU.mult)
                for half in range(2):
                    hs = slice(NG * half, NG * (half + 1))
                    ps_fr = repps.tile([T, NG], F32, tag="repps")
                    nc.tensor.matmul(ps_fr[:], lhsT=ones[0:1, 0:T],
                                     rhs=flatr[0:1, hs], start=True, stop=True)
                    nc.vector.tensor_scalar(out=m2[:, hs], in0=ps_fr[:],
                                            scalar1=ab_sb[:, 1:2], scalar2=None,
                                            op0=ALU.mult)
                nc.vector.tensor_tensor(out=tab[:], in0=m1[:], in1=m2[:], op=ALU.add)
                itw = nc.sync.dma_start(
                    out=itabs[b_][:].rearrange("(t i) c -> t i c", t=T),
                    in_=tab[:])

                # pred_polys_ table for the gt2pred gather (DRAM->DRAM via SBUF)
                pred2_b = small.tile([128, NCH, 2], F32)
                nc.sync.dma_start(
                    out=pred2_b[:],
                    in_=pred2[b_][:].rearrange("(m p) c -> p m c", m=NCH))
                ptw = nc.sync.dma_start(
                    out=ptabs[b_][:].rearrange("(m p) c -> p m c", m=NCH),
                    in_=pred2_b[:])

                # ---------- pred2gt: PE key + top-8 + exact refine ----------
                # rhs_t tiles [7, 512], shared by the 4 pred chunks
                rhs_ts = []
                for t_ in range(T):
                    rt = rhsp.tile([7, NG], F32, tag="rhs")
                    nc.vector.tensor_scalar(out=rt[:], in0=base7[:],
                                            scalar1=coef_sb[:, t_:t_ + 1],
                                            scalar2=None, op0=ALU.mult)
                    rhs_ts.append(rt)

                cand = small.tile([128, NCH, KC, 2], F32)
                gathers = []
                for m in range(NCH):
                    sl = slice(128 * m, 128 * (m + 1))
                    # partitions 0,2 <- px ; 1,3 <- py ; 4..6 <- -1
                    # (staged + single copy so the matmul has few sync waits)
                    lhsT_st = lhsp.tile([7, 128], F32, tag="lhsT_st")
                    nc.vector.memset(lhsT_st[:], -1.0)
                    nc.sync.dma_start(out=lhsT_st[0:2, :],
                                      in_=ini[b_][sl].rearrange("p c -> c p"))
                    nc.sync.dma_start(out=lhsT_st[2:4, :],
                                      in_=ini[b_][sl].rearrange("p c -> c p"))
                    lhsT = lhsp.tile([7, 128], F32, tag="lhsT")
                    nc.vector.tensor_copy(out=lhsT[:], in_=lhsT_st[:])

                    key = keyp.tile([128, NGI], F32, tag="key")
                    for t_ in range(T):
                        ps = kps.tile([128, NG], F32)
                        nc.tensor.matmul(ps[:], lhsT=lhsT[:], rhs=rhs_ts[t_][:],
                                         start=True, stop=True)
                        nc.scalar.activation(out=key[:, NG * t_:NG * (t_ + 1)],
                                             in_=ps[:], func=AF.Copy)
                    mx8 = small.tile([128, 8], F32, tag="mx8")
                    idx8 = small.tile([128, 8], U32, tag="idx8")
                    nc.vector.max(out=mx8[:], in_=key[:])
                    nc.vector.max_index(out=idx8[:], in_max=mx8[:], in_values=key[:])
                    for k in range(KC):
                        g = nc.gpsimd.indirect_dma_start(
                            out=cand[:, m, k, :], out_offset=None,
                            in_=itabs[b_][:],
                            in_offset=IndirectOffsetOnAxis(ap=idx8[:, k:k + 1],
                                                           axis=0))
                        gathers.append(g)
                for g in gathers:
                    add_dep_helper(g.ins, itw.ins, sync=True,
                                   reason="gather waits on interp table write")

                # exact refine over the KC candidates (bit-exact fp32 formula)
                pxy = small.tile([128, NCH, 2], F32)
                nc.sync.dma_start(
                    out=pxy[:], in_=ini[b_][:].rearrange("(m p) c -> p m c", m=NCH))
                dx = small.tile([128, NCH, KC], F32)
                dy = small.tile([128, NCH, KC], F32)
                nc.vector.tensor_tensor(
                    out=dx[:], in0=cand[:, :, :, 0],
                    in1=pxy[:, :, 0:1].to_broadcast([128, NCH, KC]), op=ALU.subtract)
                nc.vector.tensor_tensor(
                    out=dy[:], in0=cand[:, :, :, 1],
                    in1=pxy[:, :, 1:2].to_broadcast([128, NCH, KC]), op=ALU.subtract)
                sqx = small.tile([128, NCH, KC], F32)
                sqy = small.tile([128, NCH, KC], F32)
                dall = small.tile([128, NCH, KC], F32)
                nc.vector.tensor_tensor(out=sqx[:], in0=dx[:], in1=dx[:], op=ALU.mult)
                nc.vector.tensor_tensor(out=sqy[:], in0=dy[:], in1=dy[:], op=ALU.mult)
                nc.vector.tensor_tensor(out=dall[:], in0=sqx[:], in1=sqy[:],
                                        op=ALU.add)
                dmin = small.tile([128, NCH], F32)
                nc.vector.tensor_reduce(out=dmin[:], in_=dall[:], axis=AX.X,
                                        op=ALU.min)
                sel = small.tile([128, NCH, KC], F32)
                nc.vector.tensor_tensor(
                    out=sel[:], in0=dall[:],
                    in1=dmin[:].unsqueeze(2).to_broadcast([128, NCH, KC]),
                    op=ALU.is_equal)
                selx = small.tile([128, NCH, KC], F32)
                sely = small.tile([128, NCH, KC], F32)
                nc.vector.tensor_tensor(out=selx[:], in0=sel[:], in1=cand[:, :, :, 0],
                                        op=ALU.mult)
                nc.vector.tensor_tensor(out=sely[:], in0=sel[:], in1=cand[:, :, :, 1],
                                        op=ALU.mult)
                nx = small.tile([128, NCH], F32)
                ny = small.tile([128, NCH], F32)
                nc.vector.tensor_reduce(out=nx[:], in_=selx[:], axis=AX.X, op=ALU.add)
                nc.vector.tensor_reduce(out=ny[:], in_=sely[:], axis=AX.X, op=ALU.add)
                # |pred_polys_ - nearest_gt| partial sum -> res[:, b]
                df = small.tile([128, NCH, 2], F32)
                nc.vector.tensor_tensor(out=df[:, :, 0], in0=pred2_b[:, :, 0],
                                        in1=nx[:], op=ALU.subtract)
                nc.vector.tensor_tensor(out=df[:, :, 1], in0=pred2_b[:, :, 1],
                                        in1=ny[:], op=ALU.subtract)
                nc.vector.tensor_reduce(out=res[:, b_:b_ + 1], in_=df[:], axis=AX.XY,
                                        op=ALU.add, apply_absolute_value=True)

                # ---------- gt2pred: exact elementwise + top-1 ----------
                prow_x = g2p.tile([1, NP], F32, tag="prow_x")
                prow_y = g2p.tile([1, NP], F32, tag="prow_y")
                nc.sync.dma_start(out=prow_x[:], in_=ini[b_:b_ + 1, :, 0])
                nc.sync.dma_start(out=prow_y[:], in_=ini[b_:b_ + 1, :, 1])
                rep_px = prep.tile([128, NP], F32, tag="rep_px")
                rep_py = prep.tile([128, NP], F32, tag="rep_py")
                nc.tensor.matmul(rep_px[:], lhsT=ones[:], rhs=prow_x[:],
                                 start=True, stop=True)
                nc.tensor.matmul(rep_py[:], lhsT=ones[:], rhs=prow_y[:],
                                 start=True, stop=True)

                gt_b = small.tile([128, NCH, 2], F32, tag="gt_b")
                nc.sync.dma_start(
                    out=gt_b[:], in_=gt[b_][:].rearrange("(m p) c -> p m c", m=NCH))
                ngt = small.tile([128, NCH, 2], F32, tag="ngt")
                nc.vector.tensor_scalar(out=ngt[:], in0=gt_b[:], scalar1=-1.0,
                                        scalar2=None, op0=ALU.mult)
                mask_b = small.tile([128, NCH], F32, tag="mask_b")
                nc.sync.dma_start(
                    out=mask_b[:], in_=kmask[b_][:].rearrange("(c p) -> p c", p=128))

                npred = small.tile([128, NCH, 2], F32, tag="npred")
                g2 = []
                for c in range(NCH):
                    sq1 = g2p.tile([128, NP], F32, tag="sq1")
                    sq2 = g2p.tile([128, NP], F32, tag="sq2")
                    nc.scalar.activation(out=sq1[:], in_=rep_px[:], func=AF.Square,
                                         bias=ngt[:, c, 0:1])
                    nc.scalar.activation(out=sq2[:], in_=rep_py[:], func=AF.Square,
                                         bias=ngt[:, c, 1:2])
                    d2t = g2p.tile([128, NP], F32, tag="d2t")
                    nc.vector.tensor_tensor(out=d2t[:], in0=sq1[:], in1=sq2[:],
                                            op=ALU.add)
                    key2 = g2p.tile([128, NP], F32, tag="key2")
                    nc.vector.tensor_scalar(out=key2[:], in0=d2t[:], scalar1=-1.0,
                                            scalar2=None, op0=ALU.mult)
                    mxb = small.tile([128, 8], F32, tag="mxb")
                    ixb = small.tile([128, 8], U32, tag="ixb")
                    nc.vector.max(out=mxb[:], in_=key2[:])
                    nc.vector.max_index(out=ixb[:], in_max=mxb[:], in_values=key2[:])
                    g = nc.gpsimd.indirect_dma_start(
                        out=npred[:, c, :], out_offset=None,
                        in_=ptabs[b_][:],
                        in_offset=IndirectOffsetOnAxis(ap=ixb[:, 0:1], axis=0))
                    g2.append(g)
                for g in g2:
                    add_dep_helper(g.ins, ptw.ins, sync=True,
                                   reason="gather waits on pred table write")

                md = small.tile([128, NCH, 2], F32, tag="md")
                nc.vector.tensor_tensor(out=md[:], in0=npred[:], in1=gt_b[:],
                                        op=ALU.subtract)
                sabs = small.tile([128, NCH], F32, tag="sabs")
                nc.vector.tensor_reduce(out=sabs[:], in_=md[:], axis=AX.X,
                                        op=ALU.add, apply_absolute_value=True)
                smask = small.tile([128, NCH], F32, tag="smask")
                nc.vector.tensor_tensor(out=smask[:], in0=sabs[:], in1=mask_b[:],
                                        op=ALU.mult)
                nc.vector.tensor_reduce(out=res[:, 4 + b_:5 + b_], in_=smask[:],
                                        axis=AX.X, op=ALU.add)
                nc.vector.tensor_reduce(out=res[:, 8 + b_:9 + b_], in_=mask_b[:],
                                        axis=AX.X, op=ALU.add)

            nc.sync.dma_start(out=out[:], in_=res[:])

    nc.compile()
    return nc


_NC_CACHE = None


def _get_nc():
    global _NC_CACHE
    if _NC_CACHE is None:
        _NC_CACHE = build_nc()
    return _NC_CACHE


def make_in_maps(ini_pred_poly, pred_polys_, gt_polys, keyPointsMask):
    coef, ab = _coef_tables()
    in_maps = []
    for i in range(NCORES):
        s = slice(BLOC * i, BLOC * (i + 1))
        in_maps.append({
            "ini_pred_poly": np.ascontiguousarray(ini_pred_poly[s], dtype=np.float32),
            "pred_polys_": np.ascontiguousarray(pred_polys_[s], dtype=np.float32),
            "gt_polys": np.ascontiguousarray(gt_polys[s], dtype=np.float32),
            "keyPointsMask": np.ascontiguousarray(keyPointsMask[s], dtype=np.float32),
            "coef7": coef,
            "abcol": ab,
        })
    return in_maps


def combine_outputs(outs):
    """outs: list of [128, 12] per-core partial sums -> scalar loss (float32)."""
    acc = np.zeros(12, dtype=np.float64)
    for o in outs:
        acc += o.astype(np.float64).sum(axis=0)
    s_p2g = acc[0:4].sum()          # sum |pred_polys_ - nearest_gt|
    s_g2p = acc[4:8].sum()          # sum mask * |nearest_pred - gt|
    s_msk = 2.0 * acc[8:12].sum()   # sum of broadcast mask
    loss_pred2gt = s_p2g / (B * NP * 2)
    loss = (s_g2p / (s_msk + 1.0) + loss_pred2gt) / 2.0
    return np.float32(loss)


def kernel(ini_pred_poly, pred_polys_, gt_polys, keyPointsMask):
    nc = _get_nc()
    in_maps = make_in_maps(ini_pred_poly, pred_polys_, gt_polys, keyPointsMask)
    r = run_bass_kernel_spmd(nc, in_maps, list(range(NCORES)))
    return combine_outputs([r.results[i]["out"] for i in range(NCORES)])


if __name__ == "__main__":
    import reference

    inputs = {k: np.asarray(v) for k, v in reference.setup_inputs().items()}
    got = kernel(**inputs)
    print("kernel loss:", got)



# revision 4
# speedup vs baseline: 2.7274x; 2.7274x over previous
"""Trainium2 Bass kernel for nn_DMLoss_61942018343083 (Chamfer-style polygon
matching loss, retrieval_knn).

Sharding: data-parallel over batch B=32 across 8 NeuronCores (4 batches/core).
Each core computes partial sums into a [128, 12] output tile; the host combines
them into the scalar loss.

v2 design (vs the fp32-matmul v1):

pred2gt (argmin over 5120 interp points for each of 512 preds):
  * Ranking key v[p, (t,i)] = -d^2(p, interp(t,i)) + |p-256|^2 computed on the
    PE as a K=14 bf16 matmul per (pred-chunk, t): coordinates are recentered by
    -256 and split hi/lo into bf16 pairs (p ~ p_hi + p_lo), so each product
    p*r = p_hi*r_hi + p_hi*r_lo + p_lo*r_hi is exact to ~1 unit (lo*lo
    dropped).  bf16 matmuls run at 1 cycle/column vs fp32's 4.
  * All 14-row operand blocks are HOST-PREPARED (numpy) and DMA'd in; the
    3-per-tile packing at base partitions 0/32/64 satisfies the PE constraint
    that lhsT/rhs share a base partition in {0,32,64}.
  * t=0..5 accumulate in a 6-bank PSUM tile, reduced with one vector
    tensor_reduce(max) over a strided [128, 512, 6] view; t=6..9 drain via
    scalar ACTIVATE(Identity, bias=-|p|^2) to self-scaled bf16 and merge with
    3 vector TT(max) ops.  The [128,5120] key is never materialized.
  * MAX8/FIND_INDEX8 on the final [128,512] column-max give the best segment
    i* per pred; ONE indirect DMA per chunk gathers that segment's 10 interp
    points (host-prepped i-major table [512, 10*2]); exact fp32 refine over
    the 10 candidates picks the true nearest (CPU-sim: rel err 1.4e-4).

gt2pred (argmin over 512 preds for each of 512 gts):
  * Same trick, orientation flipped: K=8 bf16 matmul per gt-chunk
    (psum = 2*g.p - |p|^2), scalar drain with bias -|g|^2 -> bf16 -d^2 key,
    top-1 via MAX8/FIND_INDEX8, gather pred row, masked abs-diff partials.
"""

import os
import sys

for _p in ("/opt/trn_rl_repo", "/root/.axon_site/_ro/trn_rl_repo"):
    if os.path.isdir(_p) and _p not in sys.path:
        sys.path.insert(0, _p)

import numpy as np
import ml_dtypes

bfloat16 = ml_dtypes.bfloat16

import concourse.bass as bass
import concourse.bacc as bacc
import concourse.mybir as mybir
from concourse.bass import IndirectOffsetOnAxis
from concourse.bass_utils import run_bass_kernel_spmd
from concourse.tile import TileContext

F32 = mybir.dt.float32
BF16 = mybir.dt.bfloat16
U32 = mybir.dt.uint32
AF = mybir.ActivationFunctionType
ALU = mybir.AluOpType
AX = mybir.AxisListType

B, NP, NG, T = 32, 512, 512, 10
NCORES = 8
BLOC = B // NCORES          # 4 batches per core
NCH = NP // 128             # 4 chunks of 128 preds / 128 gts
CEN = np.float32(256.0)     # recentering shift
NRED = 6                    # t-banks reduced on vector; T-NRED drained on scalar
K14 = 14                    # pred2gt contraction rows
K8 = 8                      # gt2pred contraction rows


def _split_hi_lo(x):
    x = np.asarray(x, dtype=np.float32)
    hi = x.astype(bfloat16)
    lo = (x - hi.astype(np.float32)).astype(bfloat16)
    return hi, lo


def host_prep(ini_pred_poly, gt_polys):
    """Build all matmul operands / tables for one core's BLOC batches."""
    f = np.float32
    a = (np.arange(T, dtype=np.float32) / f(T)).astype(np.float32)   # t/10
    b_ = (f(1.0) - a).astype(np.float32)

    ini = np.asarray(ini_pred_poly, dtype=np.float32)   # [BLOC, NP, 2]
    gt = np.asarray(gt_polys, dtype=np.float32)         # [BLOC, NG, 2]
    gtr = np.roll(gt, 1, axis=1)

    pc = ini - CEN
    gc = gt - CEN
    gcr = np.roll(gc, 1, axis=1)

    pxh, pxl = _split_hi_lo(pc[:, :, 0])
    pyh, pyl = _split_hi_lo(pc[:, :, 1])
    m1 = np.full_like(pxh, -1.0)

    # ---- pred2gt lhs [BLOC, 78, 512] bf16 (3 copies at partitions 0/32/64)
    lhs_block = np.stack([pxh, pxh, pxh, pxh, pxl, pxl,
                          pyh, pyh, pyh, pyh, pyl, pyl, m1, m1], axis=1)  # [BLOC,14,512]
    lhs14 = np.zeros((BLOC, 78, NP), dtype=bfloat16)
    for blk in range(3):
        lhs14[:, 32 * blk:32 * blk + K14] = lhs_block

    # ---- pred2gt rhs per t: rows [r1h,r1l,r3h,r3l,r1h,r3h,r2h,r2l,r4h,r4l,r2h,r4h,ch,cl]
    u = (gc * gc).sum(-1)
    v = (gc * gcr).sum(-1)
    w = (gcr * gcr).sum(-1)
    rhs_t = np.zeros((BLOC, T, K14, NG), dtype=bfloat16)
    for t in range(T):
        r1 = (f(2.0) * a[t] * gc[:, :, 0]).astype(np.float32)
        r2 = (f(2.0) * a[t] * gc[:, :, 1]).astype(np.float32)
        r3 = (f(2.0) * b_[t] * gcr[:, :, 0]).astype(np.float32)
        r4 = (f(2.0) * b_[t] * gcr[:, :, 1]).astype(np.float32)
        c = ((a[t] * a[t] * u).astype(np.float32)
             + (f(2.0) * a[t] * b_[t] * v).astype(np.float32)
             + (b_[t] * b_[t] * w).astype(np.float32)).astype(np.float32)
        r1h, r1l = _split_hi_lo(r1)
        r2h, r2l = _split_hi_lo(r2)
        r3h, r3l = _split_hi_lo(r3)
        r4h, r4l = _split_hi_lo(r4)
        ch, cl = _split_hi_lo(c)
        rhs_t[:, t] = np.stack([r1h, r1l, r3h, r3l, r1h, r3h,
                                r2h, r2l, r4h, r4l, r2h, r4h, ch, cl], axis=1)
    # pack t=6..9 singles + t=0..5 into two padded group tensors:
    #   groups g=0,1: t=3g+blk at partition 32*blk (t 0..5)
    rhsg = np.zeros((BLOC, 2, 78, NG), dtype=bfloat16)
    for t in range(6):
        g, blk = divmod(t, 3)
        rhsg[:, g, 32 * blk:32 * blk + K14] = rhs_t[:, t]
    rhs69 = np.zeros((BLOC, 2, 78, NG), dtype=bfloat16)
    for j, t in enumerate(range(6, 10)):
        g, blk = divmod(j, 3)
        rhs69[:, g, 32 * blk:32 * blk + K14] = rhs_t[:, t]

    # ---- -|p-256|^2 per chunk [BLOC, 128, NCH] fp32
    pp = (pc * pc).sum(-1).astype(np.float32)               # [BLOC, NP]
    ppneg = (-pp).reshape(BLOC, NCH, 128).transpose(0, 2, 1).copy()

    # ---- interp table, i-major wide rows [BLOC, NG, T*2] fp32 (bit-exact ref math)
    itabw = np.empty((BLOC, NG, T, 2), dtype=np.float32)
    for t in range(T):
        itabw[:, :, t, :] = (gt * a[t]).astype(np.float32) + (gtr * (f(1.0) - a[t])).astype(np.float32)
    itabw = itabw.reshape(BLOC, NG, T * 2)

    # ---- gt2pred: lhs rows [g2xh,g2xh,g2xl,g2yh,g2yh,g2yl,m1,m1] over gts
    g2xh, g2xl = _split_hi_lo(f(2.0) * gc[:, :, 0])
    g2yh, g2yl = _split_hi_lo(f(2.0) * gc[:, :, 1])
    m1g = np.full_like(g2xh, -1.0)
    gtl8 = np.stack([g2xh, g2xh, g2xl, g2yh, g2yh, g2yl, m1g, m1g],
                    axis=1).astype(bfloat16)                 # [BLOC, 8, NG]
    pph, ppl = _split_hi_lo(pp)
    prhs8 = np.stack([pxh, pxl, pxh, pyh, pyl, pyh, pph, ppl],
                     axis=1).astype(bfloat16)                # [BLOC, 8, NP]
    ug = (gc * gc).sum(-1).astype(np.float32)                # |g-256|^2 [BLOC, NG]
    uneg = (-ug).reshape(BLOC, NCH, 128).transpose(0, 2, 1).copy()

    return dict(lhs14=lhs14, rhsg=rhsg, rhs69=rhs69, ppneg=ppneg,
                itabw=itabw, gtl8=gtl8, prhs8=prhs8, uneg=uneg)


def build_nc():
    nc = bacc.Bacc()

    ini = nc.dram_tensor("ini_pred_poly", [BLOC, NP, 2], F32, kind="ExternalInput")
    pred2 = nc.dram_tensor("pred_polys_", [BLOC, NP, 2], F32, kind="ExternalInput")
    gt = nc.dram_tensor("gt_polys", [BLOC, NG, 2], F32, kind="ExternalInput")
    kmask = nc.dram_tensor("keyPointsMask", [BLOC, NG], F32, kind="ExternalInput")
    lhs14_d = nc.dram_tensor("lhs14", [BLOC, 78, NP], BF16, kind="ExternalInput")
    rhsg_d = nc.dram_tensor("rhsg", [BLOC, 2, 78, NG], BF16, kind="ExternalInput")
    rhs69_d = nc.dram_tensor("rhs69", [BLOC, 2, 78, NG], BF16, kind="ExternalInput")
    ppneg_d = nc.dram_tensor("ppneg", [BLOC, 128, NCH], F32, kind="ExternalInput")
    gtl8_d = nc.dram_tensor("gtl8", [BLOC, K8, NG], BF16, kind="ExternalInput")
    prhs8_d = nc.dram_tensor("prhs8", [BLOC, K8, NP], BF16, kind="ExternalInput")
    uneg_d = nc.dram_tensor("uneg", [BLOC, 128, NCH], F32, kind="ExternalInput")
    # per-batch gather tables (offset-0 requirement for indirect DMA)
    itabws = [nc.dram_tensor(f"itabw{b_}", [NG, T * 2], F32, kind="ExternalInput")
              for b_ in range(BLOC)]
    ptabs = [nc.dram_tensor(f"ptab{b_}", [NP, 2], F32, kind="ExternalInput")
             for b_ in range(BLOC)]
    out = nc.dram_tensor("out", [128, 12], F32, kind="ExternalOutput")

    with TileContext(nc) as tc:
        with (
            tc.tile_pool(name="const", bufs=1) as cpool,
            tc.tile_pool(name="bat", bufs=2) as bat,
            tc.tile_pool(name="drain", bufs=3) as drp,
            tc.tile_pool(name="mrg", bufs=2) as mrg,
            tc.tile_pool(name="small", bufs=2) as small,
            tc.tile_pool(name="psA", bufs=1, space="PSUM") as psap,
            tc.tile_pool(name="psT", bufs=2, space="PSUM") as pstp,
        ):
            res = cpool.tile([128, 12], F32)
            nc.vector.memset(res[:], 0.0)
            ppneg_sb = cpool.tile([128, BLOC, NCH], F32)
            nc.sync.dma_start(out=ppneg_sb[:],
                              in_=ppneg_d[:].rearrange("b p c -> p b c"))
            uneg_sb = cpool.tile([128, BLOC, NCH], F32)
            nc.sync.dma_start(out=uneg_sb[:],
                              in_=uneg_d[:].rearrange("b p c -> p b c"))

            for b_ in range(BLOC):
                # ---------------- batch-level loads ----------------
                lhs14 = bat.tile([78, NP], BF16, tag="lhs14")
                nc.sync.dma_start(out=lhs14[:], in_=lhs14_d[b_])
                rhsg0 = bat.tile([78, NG], BF16, tag="rhsg0")
                rhsg1 = bat.tile([78, NG], BF16, tag="rhsg1")
                nc.scalar.dma_start(out=rhsg0[:], in_=rhsg_d[b_, 0])
                nc.scalar.dma_start(out=rhsg1[:], in_=rhsg_d[b_, 1])
                rhs690 = bat.tile([78, NG], BF16, tag="rhs690")
                rhs691 = bat.tile([78, NG], BF16, tag="rhs691")
                nc.scalar.dma_start(out=rhs690[:], in_=rhs69_d[b_, 0])
                nc.scalar.dma_start(out=rhs691[:], in_=rhs69_d[b_, 1])
                gtl8 = bat.tile([K8, NG], BF16, tag="gtl8")
                nc.scalar.dma_start(out=gtl8[:], in_=gtl8_d[b_])
                prhs8 = bat.tile([K8, NP], BF16, tag="prhs8")
                nc.scalar.dma_start(out=prhs8[:], in_=prhs8_d[b_])
                pxy = bat.tile([128, NCH, 2], F32, tag="pxy")
                nc.sync.dma_start(
                    out=pxy[:], in_=ini[b_][:].rearrange("(m p) c -> p m c", m=NCH))
                pred2_b = bat.tile([128, NCH, 2], F32, tag="pred2_b")
                nc.sync.dma_start(
                    out=pred2_b[:],
                    in_=pred2[b_][:].rearrange("(m p) c -> p m c", m=NCH))
                gt_b = bat.tile([128, NCH, 2], F32, tag="gt_b")
                nc.sync.dma_start(
                    out=gt_b[:], in_=gt[b_][:].rearrange("(m p) c -> p m c", m=NCH))
                mask_b = bat.tile([128, NCH], F32, tag="mask_b")
                nc.sync.dma_start(
                    out=mask_b[:], in_=kmask[b_][:].rearrange("(c p) -> p c", p=128))

                cand = bat.tile([128, NCH, T, 2], F32, tag="cand")
                npred = bat.tile([128, NCH, 2], F32, tag="npred")

                def rhs_view(t):
                    if t < 6:
                        g, blk = divmod(t, 3)
                        tile = rhsg0 if g == 0 else rhsg1
                    else:
                        g, blk = divmod(t - 6, 3)
                        tile = rhs690 if g == 0 else rhs691
                    return tile[32 * blk:32 * blk + K14, :], 32 * blk

                # ---------------- pred2gt ----------------
                for m in range(NCH):
                    sl = slice(128 * m, 128 * (m + 1))
                    psA = psap.tile([128, NRED, NG], F32, tag="psA")
                    for t in range(NRED):
                        rv, base = rhs_view(t)
                        nc.tensor.matmul(psA[:, t, :],
                                         lhsT=lhs14[base:base + K14, sl],
                                         rhs=rv, start=True, stop=True)
                    vs = []
                    for t in range(NRED, T):
                        rv, base = rhs_view(t)
                        pst = pstp.tile([128, NG], F32, tag="psT")
                        nc.tensor.matmul(pst[:], lhsT=lhs14[base:base + K14, sl],
                                         rhs=rv, start=True, stop=True)
                        vt = drp.tile([128, NG], BF16, tag="vt")
                        nc.scalar.activation(out=vt[:], in_=pst[:], func=AF.Identity,
                                             bias=ppneg_sb[:, b_, m:m + 1])
                        vs.append(vt)
                    # vector: reduce 6 banks + merge 4 bf16 drains
                    rA = mrg.tile([128, NG], F32, tag="rA")
                    nc.vector.tensor_reduce(out=rA[:],
                                            in_=psA[:].rearrange("p t i -> p i t"),
                                            axis=AX.X, op=ALU.max)
                    m1_ = mrg.tile([128, NG], BF16, tag="m1_")
                    nc.vector.tensor_tensor(out=m1_[:], in0=vs[0][:], in1=vs[1][:],
                                            op=ALU.max)
                    m2_ = mrg.tile([128, NG], BF16, tag="m2_")
                    nc.vector.tensor_tensor(out=m2_[:], in0=vs[2][:], in1=vs[3][:],
                                            op=ALU.max)
                    m3_ = mrg.tile([128, NG], BF16, tag="m3_")
                    nc.vector.tensor_tensor(out=m3_[:], in0=m1_[:], in1=m2_[:],
                                            op=ALU.max)
                    vkey = mrg.tile([128, NG], BF16, tag="vkey")
                    nc.vector.scalar_tensor_tensor(
                        out=vkey[:], in0=rA[:], scalar=ppneg_sb[:, b_, m:m + 1],
                        in1=m3_[:], op0=ALU.add, op1=ALU.max)
                    mx8 = small.tile([128, 8], BF16, tag="mx8")
                    i8 = small.tile([128, 8], U32, tag="i8")
                    nc.vector.max(out=mx8[:], in_=vkey[:])
                    nc.vector.max_index(out=i8[:], in_max=mx8[:], in_values=vkey[:])
                    nc.gpsimd.indirect_dma_start(
                        out=cand[:, m].rearrange("p t c -> p (t c)"),
                        out_offset=None, in_=itabws[b_][:],
                        in_offset=IndirectOffsetOnAxis(ap=i8[:, 0:1], axis=0))

                # exact fp32 refine over the 10 candidates of the winning segment
                dx = small.tile([128, NCH, T], F32, tag="dx")
                dy = small.tile([128, NCH, T], F32, tag="dy")
                nc.vector.tensor_tensor(
                    out=dx[:], in0=cand[:, :, :, 0],
                    in1=pxy[:, :, 0:1].to_broadcast([128, NCH, T]), op=ALU.subtract)
                nc.vector.tensor_tensor(
                    out=dy[:], in0=cand[:, :, :, 1],
                    in1=pxy[:, :, 1:2].to_broadcast([128, NCH, T]), op=ALU.subtract)
                sqx = small.tile([128, NCH, T], F32, tag="sqx")
                sqy = small.tile([128, NCH, T], F32, tag="sqy")
                dall = small.tile([128, NCH, T], F32, tag="dall")
                nc.vector.tensor_tensor(out=sqx[:], in0=dx[:], in1=dx[:], op=ALU.mult)
                nc.vector.tensor_tensor(out=sqy[:], in0=dy[:], in1=dy[:], op=ALU.mult)
                nc.vector.tensor_tensor(out=dall[:], in0=sqx[:], in1=sqy[:],
                                        op=ALU.add)
                dmin = small.tile([128, NCH], F32, tag="dmin")
                nc.vector.tensor_reduce(out=dmin[:], in_=dall[:], axis=AX.X,
                                        op=ALU.min)
                sel = small.tile([128, NCH, T], F32, tag="sel")
                nc.vector.tensor_tensor(
                    out=sel[:], in0=dall[:],
                    in1=dmin[:].unsqueeze(2).to_broadcast([128, NCH, T]),
                    op=ALU.is_equal)
                selx = small.tile([128, NCH, T], F32, tag="selx")
                sely = small.tile([128, NCH, T], F32, tag="sely")
                nc.vector.tensor_tensor(out=selx[:], in0=sel[:], in1=cand[:, :, :, 0],
                                        op=ALU.mult)
                nc.vector.tensor_tensor(out=sely[:], in0=sel[:], in1=cand[:, :, :, 1],
                                        op=ALU.mult)
                nx = small.tile([128, NCH], F32, tag="nx")
                ny = small.tile([128, NCH], F32, tag="ny")
                cnt = small.tile([128, NCH], F32, tag="cnt")
                nc.vector.tensor_reduce(out=nx[:], in_=selx[:], axis=AX.X, op=ALU.add)
                nc.vector.tensor_reduce(out=ny[:], in_=sely[:], axis=AX.X, op=ALU.add)
                nc.vector.tensor_reduce(out=cnt[:], in_=sel[:], axis=AX.X, op=ALU.add)
                rcnt = small.tile([128, NCH], F32, tag="rcnt")
                nc.vector.reciprocal(out=rcnt[:], in_=cnt[:])
                nxn = small.tile([128, NCH], F32, tag="nxn")
                nyn = small.tile([128, NCH], F32, tag="nyn")
                nc.vector.tensor_tensor(out=nxn[:], in0=nx[:], in1=rcnt[:], op=ALU.mult)
                nc.vector.tensor_tensor(out=nyn[:], in0=ny[:], in1=rcnt[:], op=ALU.mult)
                df = small.tile([128, NCH, 2], F32, tag="df")
                nc.vector.tensor_tensor(out=df[:, :, 0], in0=pred2_b[:, :, 0],
                                        in1=nxn[:], op=ALU.subtract)
                nc.vector.tensor_tensor(out=df[:, :, 1], in0=pred2_b[:, :, 1],
                                        in1=nyn[:], op=ALU.subtract)
                nc.vector.tensor_reduce(out=res[:, b_:b_ + 1], in_=df[:], axis=AX.XY,
                                        op=ALU.add, apply_absolute_value=True)

                # ---------------- gt2pred ----------------
                for c in range(NCH):
                    sl = slice(128 * c, 128 * (c + 1))
                    ps2 = pstp.tile([128, NP], F32, tag="psT")
                    nc.tensor.matmul(ps2[:], lhsT=gtl8[:, sl], rhs=prhs8[:],
                                     start=True, stop=True)
                    key2 = drp.tile([128, NP], BF16, tag="key2")
                    nc.scalar.activation(out=key2[:], in_=ps2[:], func=AF.Identity,
                                         bias=uneg_sb[:, b_, c:c + 1])
                    mxb = small.tile([128, 8], BF16, tag="mxb")
                    ixb = small.tile([128, 8], U32, tag="ixb")
                    nc.vector.max(out=mxb[:], in_=key2[:])
                    nc.vector.max_index(out=ixb[:], in_max=mxb[:], in_values=key2[:])
                    nc.gpsimd.indirect_dma_start(
                        out=npred[:, c, :], out_offset=None,
                        in_=ptabs[b_][:],
                        in_offset=IndirectOffsetOnAxis(ap=ixb[:, 0:1], axis=0))

                md = small.tile([128, NCH, 2], F32, tag="md")
                nc.vector.tensor_tensor(out=md[:], in0=npred[:], in1=gt_b[:],
                                        op=ALU.subtract)
                sabs = small.tile([128, NCH], F32, tag="sabs")
                nc.vector.tensor_reduce(out=sabs[:], in_=md[:], axis=AX.X,
                                        op=ALU.add, apply_absolute_value=True)
                smask = small.tile([128, NCH], F32, tag="smask")
                nc.vector.tensor_tensor(out=smask[:], in0=sabs[:], in1=mask_b[:],
                                        op=ALU.mult)
                nc.vector.tensor_reduce(out=res[:, 4 + b_:5 + b_], in_=smask[:],
                                        axis=AX.X, op=ALU.add)
                nc.vector.tensor_reduce(out=res[:, 8 + b_:9 + b_], in_=mask_b[:],
                                        axis=AX.X, op=ALU.add)

            nc.sync.dma_start(out=out[:], in_=res[:])

    nc.compile()
    return nc


_NC_CACHE = None


def _get_nc():
    global _NC_CACHE
    if _NC_CACHE is None:
        _NC_CACHE = build_nc()
    return _NC_CACHE


def make_in_maps(ini_pred_poly, pred_polys_, gt_polys, keyPointsMask):
    in_maps = []
    for i in range(NCORES):
        s = slice(BLOC * i, BLOC * (i + 1))
        ini = np.ascontiguousarray(ini_pred_poly[s], dtype=np.float32)
        p2 = np.ascontiguousarray(pred_polys_[s], dtype=np.float32)
        gp = np.ascontiguousarray(gt_polys[s], dtype=np.float32)
        km = np.ascontiguousarray(keyPointsMask[s], dtype=np.float32)
        hp = host_prep(ini, gp)
        im = {
            "ini_pred_poly": ini,
            "pred_polys_": p2,
            "gt_polys": gp,
            "keyPointsMask": km,
            "lhs14": hp["lhs14"],
            "rhsg": hp["rhsg"],
            "rhs69": hp["rhs69"],
            "ppneg": hp["ppneg"],
            "gtl8": hp["gtl8"],
            "prhs8": hp["prhs8"],
            "uneg": hp["uneg"],
        }
        for b_ in range(BLOC):
            im[f"itabw{b_}"] = np.ascontiguousarray(hp["itabw"][b_])
            im[f"ptab{b_}"] = np.ascontiguousarray(p2[b_])
        in_maps.append(im)
    return in_maps


def combine_outputs(outs):
    """outs: list of [128, 12] per-core partial sums -> scalar loss (float32)."""
    acc = np.zeros(12, dtype=np.float64)
    for o in outs:
        acc += o.astype(np.float64).sum(axis=0)
    s_p2g = acc[0:4].sum()          # sum |pred_polys_ - nearest_gt|
    s_g2p = acc[4:8].sum()          # sum mask * |nearest_pred - gt|
    s_msk = 2.0 * acc[8:12].sum()   # sum of broadcast mask
    loss_pred2gt = s_p2g / (B * NP * 2)
    loss = (s_g2p / (s_msk + 1.0) + loss_pred2gt) / 2.0
    return np.float32(loss)


def kernel(ini_pred_poly, pred_polys_, gt_polys, keyPointsMask):
    nc = _get_nc()
    in_maps = make_in_maps(ini_pred_poly, pred_polys_, gt_polys, keyPointsMask)
    r = run_bass_kernel_spmd(nc, in_maps, list(range(NCORES)))
    return combine_outputs([r.results[i]["out"] for i in range(NCORES)])


if __name__ == "__main__":
    import reference

    inputs = {k: np.asarray(v) for k, v in reference.setup_inputs().items()}
    got = kernel(**inputs)
    print("kernel loss:", got)


# revision 16
# speedup vs baseline: 3.2408x; 1.1882x over previous
"""Trainium2 Bass kernel for nn_DMLoss_61942018343083 (Chamfer-style polygon
matching loss, retrieval_knn).

Sharding: data-parallel over batch B=32 across 8 NeuronCores (4 batches/core).
Each core computes partial sums into a [128, 12] output tile; the host combines
them into the scalar loss.

v2 design (vs the fp32-matmul v1):

pred2gt (argmin over 5120 interp points for each of 512 preds):
  * Ranking key v[p, (t,i)] = -d^2(p, interp(t,i)) + |p-256|^2 computed on the
    PE as a K=14 bf16 matmul per (pred-chunk, t): coordinates are recentered by
    -256 and split hi/lo into bf16 pairs (p ~ p_hi + p_lo), so each product
    p*r = p_hi*r_hi + p_hi*r_lo + p_lo*r_hi is exact to ~1 unit (lo*lo
    dropped).  bf16 matmuls run at 1 cycle/column vs fp32's 4.
  * All 14-row operand blocks are HOST-PREPARED (numpy) and DMA'd in; the
    3-per-tile packing at base partitions 0/32/64 satisfies the PE constraint
    that lhsT/rhs share a base partition in {0,32,64}.
  * t=0..5 accumulate in a 6-bank PSUM tile, reduced with one vector
    tensor_reduce(max) over a strided [128, 512, 6] view; t=6..9 drain via
    scalar ACTIVATE(Identity, bias=-|p|^2) to self-scaled bf16 and merge with
    3 vector TT(max) ops.  The [128,5120] key is never materialized.
  * MAX8/FIND_INDEX8 on the final [128,512] column-max give the best segment
    i* per pred; ONE indirect DMA per chunk gathers that segment's 10 interp
    points (host-prepped i-major table [512, 10*2]); exact fp32 refine over
    the 10 candidates picks the true nearest (CPU-sim: rel err 1.4e-4).

gt2pred (argmin over 512 preds for each of 512 gts):
  * Same trick, orientation flipped: K=8 bf16 matmul per gt-chunk
    (psum = 2*g.p - |p|^2), scalar drain with bias -|g|^2 -> bf16 -d^2 key,
    top-1 via MAX8/FIND_INDEX8, gather pred row, masked abs-diff partials.
"""

import os
import sys

for _p in ("/opt/trn_rl_repo", "/root/.axon_site/_ro/trn_rl_repo"):
    if os.path.isdir(_p) and _p not in sys.path:
        sys.path.insert(0, _p)

import numpy as np
import ml_dtypes

bfloat16 = ml_dtypes.bfloat16

import concourse.bass as bass
import concourse.bacc as bacc
import concourse.mybir as mybir
from concourse.bass import IndirectOffsetOnAxis
from concourse.bass_utils import run_bass_kernel_spmd
from concourse.tile import TileContext

F32 = mybir.dt.float32
BF16 = mybir.dt.bfloat16
U32 = mybir.dt.uint32
AF = mybir.ActivationFunctionType
ALU = mybir.AluOpType
AX = mybir.AxisListType

B, NP, NG, T = 32, 512, 512, 10
NCORES = 8
BLOC = B // NCORES          # 4 batches per core
NCH = NP // 128             # 4 chunks of 128 preds / 128 gts
CEN = np.float32(256.0)     # recentering shift
NRED = 6                    # t-banks reduced on vector; T-NRED drained on scalar
K14 = 14                    # pred2gt contraction rows
K8 = 8                      # gt2pred contraction rows


def _split_hi_lo(x):
    x = np.asarray(x, dtype=np.float32)
    hi = x.astype(bfloat16)
    lo = (x - hi.astype(np.float32)).astype(bfloat16)
    return hi, lo


def host_prep(ini_pred_poly, gt_polys):
    """Build all matmul operands / tables for one core's BLOC batches."""
    f = np.float32
    a = (np.arange(T, dtype=np.float32) / f(T)).astype(np.float32)   # t/10
    b_ = (f(1.0) - a).astype(np.float32)

    ini = np.asarray(ini_pred_poly, dtype=np.float32)   # [BLOC, NP, 2]
    gt = np.asarray(gt_polys, dtype=np.float32)         # [BLOC, NG, 2]
    gtr = np.roll(gt, 1, axis=1)

    pc = ini - CEN
    gc = gt - CEN
    gcr = np.roll(gc, 1, axis=1)

    pxh, pxl = _split_hi_lo(pc[:, :, 0])
    pyh, pyl = _split_hi_lo(pc[:, :, 1])
    m1 = np.full_like(pxh, -1.0)

    # ---- pred2gt projection operands ----
    # Q[p,i] = (p - g_{i-1}) . s_i    (s = g_i - g_{i-1})
    # R[p,i] = -|p - g_{i-1}|^2  =  2 p.g_ - |g_|^2 - |p|^2   (centered coords)
    pp = (pc * pc).sum(-1).astype(np.float32)               # |p-256|^2 [BLOC, NP]
    s = (gc - gcr).astype(np.float32)
    len2 = (s * s).sum(-1).astype(np.float32)
    inv10 = np.where(len2 > 1e-6, (f(10.0) / len2).astype(np.float32),
                     np.float32(0.0)).astype(np.float32)
    negalpha = (-(len2 / f(100.0))).astype(np.float32)
    gs_ = (gcr * s).sum(-1).astype(np.float32)
    ug_ = (gcr * gcr).sum(-1).astype(np.float32)

    one = np.ones_like(pxh, dtype=np.float32)
    sxh, sxl = _split_hi_lo(s[:, :, 0])
    syh, syl = _split_hi_lo(s[:, :, 1])
    gsh, gsl = _split_hi_lo(gs_)
    lhsQ = np.stack([pxh, pxh, pxl, pyh, pyh, pyl, m1, m1],
                    axis=1).astype(bfloat16)                  # [BLOC, 8, NP]
    rhsQ = np.stack([sxh, sxl, sxh, syh, syl, syh, gsh, gsl],
                    axis=1).astype(bfloat16)                  # [BLOC, 8, NG]

    g2rxh, g2rxl = _split_hi_lo(f(2.0) * gcr[:, :, 0])
    g2ryh, g2ryl = _split_hi_lo(f(2.0) * gcr[:, :, 1])
    ugh, ugl = _split_hi_lo(ug_)
    npph, nppl = _split_hi_lo(-pp)
    oneb = one.astype(bfloat16)
    lhsR = np.stack([pxh, pxh, pxl, pyh, pyh, pyl, m1, m1, npph, nppl],
                    axis=1).astype(bfloat16)                  # [BLOC, 10, NP]
    rhsR = np.stack([g2rxh, g2rxl, g2rxh, g2ryh, g2ryl, g2ryh, ugh, ugl,
                     oneb, oneb], axis=1).astype(bfloat16)    # [BLOC, 10, NG]
    # per-column broadcast tiles (replicated across 128 partitions host-side)
    inv10_b = np.broadcast_to(inv10[:, None, :], (BLOC, 128, NG)).copy()
    negal_b = np.broadcast_to(negalpha[:, None, :], (BLOC, 128, NG)).copy()

    # ---- interp table, i-major wide rows [BLOC, NG, T*2] fp32 (bit-exact ref math)
    itabw = np.empty((BLOC, NG, T, 2), dtype=np.float32)
    for t in range(T):
        itabw[:, :, t, :] = (gt * a[t]).astype(np.float32) + (gtr * (f(1.0) - a[t])).astype(np.float32)
    itabw = itabw.reshape(BLOC, NG, T * 2)

    # ---- gt2pred: lhs rows [g2xh,g2xh,g2xl,g2yh,g2yh,g2yl,m1,m1] over gts
    g2xh, g2xl = _split_hi_lo(f(2.0) * gc[:, :, 0])
    g2yh, g2yl = _split_hi_lo(f(2.0) * gc[:, :, 1])
    m1g = np.full_like(g2xh, -1.0)
    gtl8 = np.stack([g2xh, g2xh, g2xl, g2yh, g2yh, g2yl, m1g, m1g],
                    axis=1).astype(bfloat16)                 # [BLOC, 8, NG]
    pph, ppl = _split_hi_lo(pp)
    prhs8 = np.stack([pxh, pxl, pxh, pyh, pyl, pyh, pph, ppl],
                     axis=1).astype(bfloat16)                # [BLOC, 8, NP]
    ug = (gc * gc).sum(-1).astype(np.float32)                # |g-256|^2 [BLOC, NG]
    uneg = (-ug).reshape(BLOC, NCH, 128).transpose(0, 2, 1).copy()

    return dict(lhsQ=lhsQ, rhsQ=rhsQ, lhsR=lhsR, rhsR=rhsR,
                inv10_b=inv10_b, negal_b=negal_b,
                itabw=itabw, gtl8=gtl8, prhs8=prhs8, uneg=uneg)


def build_nc():
    nc = bacc.Bacc()

    ini = nc.dram_tensor("ini_pred_poly", [BLOC, NP, 2], F32, kind="ExternalInput")
    pred2 = nc.dram_tensor("pred_polys_", [BLOC, NP, 2], F32, kind="ExternalInput")
    gt = nc.dram_tensor("gt_polys", [BLOC, NG, 2], F32, kind="ExternalInput")
    kmask = nc.dram_tensor("keyPointsMask", [BLOC, NG], F32, kind="ExternalInput")
    lhsQ_d = nc.dram_tensor("lhsQ", [BLOC, K8, NP], BF16, kind="ExternalInput")
    rhsQ_d = nc.dram_tensor("rhsQ", [BLOC, K8, NG], BF16, kind="ExternalInput")
    lhsR_d = nc.dram_tensor("lhsR", [BLOC, 10, NP], BF16, kind="ExternalInput")
    rhsR_d = nc.dram_tensor("rhsR", [BLOC, 10, NG], BF16, kind="ExternalInput")
    inv10_d = nc.dram_tensor("inv10_b", [BLOC, 128, NG], F32, kind="ExternalInput")
    negal_d = nc.dram_tensor("negal_b", [BLOC, 128, NG], F32, kind="ExternalInput")
    gtl8_d = nc.dram_tensor("gtl8", [BLOC, K8, NG], BF16, kind="ExternalInput")
    prhs8_d = nc.dram_tensor("prhs8", [BLOC, K8, NP], BF16, kind="ExternalInput")
    uneg_d = nc.dram_tensor("uneg", [BLOC, 128, NCH], F32, kind="ExternalInput")
    # per-batch gather tables (offset-0 requirement for indirect DMA)
    itabws = [nc.dram_tensor(f"itabw{b_}", [NG, T * 2], F32, kind="ExternalInput")
              for b_ in range(BLOC)]
    ptabs = [nc.dram_tensor(f"ptab{b_}", [NP, 2], F32, kind="ExternalInput")
             for b_ in range(BLOC)]
    out = nc.dram_tensor("out", [128, 12], F32, kind="ExternalOutput")

    with TileContext(nc) as tc:
        with (
            tc.tile_pool(name="const", bufs=1) as cpool,
            tc.tile_pool(name="bat", bufs=2) as bat,
            tc.tile_pool(name="drain", bufs=3) as drp,
            tc.tile_pool(name="mrg", bufs=2) as mrg,
            tc.tile_pool(name="small", bufs=2) as small,
            tc.tile_pool(name="psA", bufs=2, space="PSUM") as psap,
            tc.tile_pool(name="psT", bufs=2, space="PSUM") as pstp,
        ):
            res = cpool.tile([128, 12], F32)
            nc.vector.memset(res[:], 0.0)
            c9 = cpool.tile([128, 1], F32)
            nc.vector.memset(c9[:], 9.0)
            uneg_sb = cpool.tile([128, BLOC, NCH], F32)
            nc.sync.dma_start(out=uneg_sb[:],
                              in_=uneg_d[:].rearrange("b p c -> p b c"))

            for b_ in range(BLOC):
                # ---------------- batch-level loads ----------------
                lhsQ = bat.tile([K8, NP], BF16, tag="lhsQ")
                nc.sync.dma_start(out=lhsQ[:], in_=lhsQ_d[b_])
                rhsQ = bat.tile([K8, NG], BF16, tag="rhsQ")
                nc.sync.dma_start(out=rhsQ[:], in_=rhsQ_d[b_])
                lhsR = bat.tile([10, NP], BF16, tag="lhsR")
                nc.scalar.dma_start(out=lhsR[:], in_=lhsR_d[b_])
                rhsR = bat.tile([10, NG], BF16, tag="rhsR")
                nc.scalar.dma_start(out=rhsR[:], in_=rhsR_d[b_])
                inv10_b = bat.tile([128, NG], F32, tag="inv10_b")
                nc.scalar.dma_start(out=inv10_b[:], in_=inv10_d[b_])
                negal_b = bat.tile([128, NG], F32, tag="negal_b")
                nc.scalar.dma_start(out=negal_b[:], in_=negal_d[b_])
                gtl8 = bat.tile([K8, NG], BF16, tag="gtl8")
                nc.scalar.dma_start(out=gtl8[:], in_=gtl8_d[b_])
                prhs8 = bat.tile([K8, NP], BF16, tag="prhs8")
                nc.scalar.dma_start(out=prhs8[:], in_=prhs8_d[b_])
                pxy = bat.tile([128, NCH, 2], F32, tag="pxy")
                nc.sync.dma_start(
                    out=pxy[:], in_=ini[b_][:].rearrange("(m p) c -> p m c", m=NCH))
                pred2_b = bat.tile([128, NCH, 2], F32, tag="pred2_b")
                nc.sync.dma_start(
                    out=pred2_b[:],
                    in_=pred2[b_][:].rearrange("(m p) c -> p m c", m=NCH))
                gt_b = bat.tile([128, NCH, 2], F32, tag="gt_b")
                nc.sync.dma_start(
                    out=gt_b[:], in_=gt[b_][:].rearrange("(m p) c -> p m c", m=NCH))
                mask_b = bat.tile([128, NCH], F32, tag="mask_b")
                nc.sync.dma_start(
                    out=mask_b[:], in_=kmask[b_][:].rearrange("(c p) -> p c", p=128))

                cand = bat.tile([128, NCH, T, 2], F32, tag="cand")
                npred = bat.tile([128, NCH, 2], F32, tag="npred")

                # ---------------- pred2gt (point-to-segment projection) ------
                # Q = w.s in PSUM, R = -|w|^2 in PSUM (w = p - g_{i-1}).
                # t* = clamp(round(10 Q / len2), 0, 9) via Relu-chain on scalar
                # vkey = R + t*(0.2 Q - (len2/100) t*) = -d^2(t*)  (fp32)
                for m in range(NCH):
                    sl = slice(128 * m, 128 * (m + 1))
                    psQ = psap.tile([128, NG], F32, tag="psQ")
                    nc.tensor.matmul(psQ[:], lhsT=lhsQ[:, sl], rhs=rhsQ[:],
                                     start=True, stop=True)
                    psR = psap.tile([128, NG], F32, tag="psR")
                    nc.tensor.matmul(psR[:], lhsT=lhsR[:, sl], rhs=rhsR[:],
                                     start=True, stop=True)
                    z = mrg.tile([128, NG], F32, tag="z")
                    nc.vector.tensor_tensor(out=z[:], in0=psQ[:], in1=inv10_b[:],
                                            op=ALU.mult)
                    # clamp on the scalar engine: y = min(max(z,0),9), then
                    # round-to-nearest via the fp32 magic constant on vector
                    r1 = drp.tile([128, NG], F32, tag="r1")
                    nc.scalar.activation(out=r1[:], in_=z[:], func=AF.Relu)
                    r2 = drp.tile([128, NG], F32, tag="r2")
                    nc.scalar.activation(out=r2[:], in_=r1[:], func=AF.Relu,
                                         bias=c9[:, 0:1], scale=-1.0)
                    y = drp.tile([128, NG], F32, tag="y")
                    nc.scalar.activation(out=y[:], in_=r2[:], func=AF.Copy,
                                         bias=9.0, scale=-1.0)
                    ymag = mrg.tile([128, NG], F32, tag="ymag")
                    nc.vector.tensor_scalar(out=ymag[:], in0=y[:], scalar1=8388608.0,
                                            scalar2=None, op0=ALU.add)
                    ts_ = mrg.tile([128, NG], F32, tag="ts_")
                    nc.vector.tensor_scalar(out=ts_[:], in0=ymag[:], scalar1=8388608.0,
                                            scalar2=None, op0=ALU.subtract)
                    w1 = mrg.tile([128, NG], F32, tag="w1")
                    nc.vector.tensor_tensor(out=w1[:], in0=negal_b[:], in1=ts_[:],
                                            op=ALU.mult)
                    w2 = mrg.tile([128, NG], F32, tag="w2")
                    nc.vector.scalar_tensor_tensor(out=w2[:], in0=psQ[:], scalar=0.2,
                                                   in1=w1[:], op0=ALU.mult,
                                                   op1=ALU.add)
                    w3 = mrg.tile([128, NG], F32, tag="w3")
                    nc.vector.tensor_tensor(out=w3[:], in0=w2[:], in1=ts_[:],
                                            op=ALU.mult)
                    vkey = mrg.tile([128, NG], F32, tag="vkey")
                    nc.vector.scalar_tensor_tensor(out=vkey[:], in0=psR[:], scalar=1.0,
                                                   in1=w3[:], op0=ALU.mult,
                                                   op1=ALU.add)
                    mx8 = small.tile([128, 8], F32, tag="mx8")
                    i8 = small.tile([128, 8], U32, tag="i8")
                    nc.vector.max(out=mx8[:], in_=vkey[:])
                    nc.vector.max_index(out=i8[:], in_max=mx8[:], in_values=vkey[:])
                    nc.gpsimd.indirect_dma_start(
                        out=cand[:, m].rearrange("p t c -> p (t c)"),
                        out_offset=None, in_=itabws[b_][:],
                        in_offset=IndirectOffsetOnAxis(ap=i8[:, 0:1], axis=0))

                # exact fp32 refine over the 10 candidates of the winning segment
                dx = small.tile([128, NCH, T], F32, tag="dx")
                dy = small.tile([128, NCH, T], F32, tag="dy")
                nc.vector.tensor_tensor(
                    out=dx[:], in0=cand[:, :, :, 0],
                    in1=pxy[:, :, 0:1].to_broadcast([128, NCH, T]), op=ALU.subtract)
                nc.vector.tensor_tensor(
                    out=dy[:], in0=cand[:, :, :, 1],
                    in1=pxy[:, :, 1:2].to_broadcast([128, NCH, T]), op=ALU.subtract)
                sqx = small.tile([128, NCH, T], F32, tag="sqx")
                sqy = small.tile([128, NCH, T], F32, tag="sqy")
                dall = small.tile([128, NCH, T], F32, tag="dall")
                nc.vector.tensor_tensor(out=sqx[:], in0=dx[:], in1=dx[:], op=ALU.mult)
                nc.vector.tensor_tensor(out=sqy[:], in0=dy[:], in1=dy[:], op=ALU.mult)
                nc.vector.tensor_tensor(out=dall[:], in0=sqx[:], in1=sqy[:],
                                        op=ALU.add)
                dmin = small.tile([128, NCH], F32, tag="dmin")
                nc.vector.tensor_reduce(out=dmin[:], in_=dall[:], axis=AX.X,
                                        op=ALU.min)
                sel = small.tile([128, NCH, T], F32, tag="sel")
                nc.vector.tensor_tensor(
                    out=sel[:], in0=dall[:],
                    in1=dmin[:].unsqueeze(2).to_broadcast([128, NCH, T]),
                    op=ALU.is_equal)
                selx = small.tile([128, NCH, T], F32, tag="selx")
                sely = small.tile([128, NCH, T], F32, tag="sely")
                nc.vector.tensor_tensor(out=selx[:], in0=sel[:], in1=cand[:, :, :, 0],
                                        op=ALU.mult)
                nc.vector.tensor_tensor(out=sely[:], in0=sel[:], in1=cand[:, :, :, 1],
                                        op=ALU.mult)
                nx = small.tile([128, NCH], F32, tag="nx")
                ny = small.tile([128, NCH], F32, tag="ny")
                cnt = small.tile([128, NCH], F32, tag="cnt")
                nc.vector.tensor_reduce(out=nx[:], in_=selx[:], axis=AX.X, op=ALU.add)
                nc.vector.tensor_reduce(out=ny[:], in_=sely[:], axis=AX.X, op=ALU.add)
                nc.vector.tensor_reduce(out=cnt[:], in_=sel[:], axis=AX.X, op=ALU.add)
                rcnt = small.tile([128, NCH], F32, tag="rcnt")
                nc.vector.reciprocal(out=rcnt[:], in_=cnt[:])
                nxn = small.tile([128, NCH], F32, tag="nxn")
                nyn = small.tile([128, NCH], F32, tag="nyn")
                nc.vector.tensor_tensor(out=nxn[:], in0=nx[:], in1=rcnt[:], op=ALU.mult)
                nc.vector.tensor_tensor(out=nyn[:], in0=ny[:], in1=rcnt[:], op=ALU.mult)
                df = small.tile([128, NCH, 2], F32, tag="df")
                nc.vector.tensor_tensor(out=df[:, :, 0], in0=pred2_b[:, :, 0],
                                        in1=nxn[:], op=ALU.subtract)
                nc.vector.tensor_tensor(out=df[:, :, 1], in0=pred2_b[:, :, 1],
                                        in1=nyn[:], op=ALU.subtract)
                nc.vector.tensor_reduce(out=res[:, b_:b_ + 1], in_=df[:], axis=AX.XY,
                                        op=ALU.add, apply_absolute_value=True)

                # ---------------- gt2pred ----------------
                for c in range(NCH):
                    sl = slice(128 * c, 128 * (c + 1))
                    ps2 = pstp.tile([128, NP], F32, tag="psT")
                    nc.tensor.matmul(ps2[:], lhsT=gtl8[:, sl], rhs=prhs8[:],
                                     start=True, stop=True)
                    key2 = drp.tile([128, NP], BF16, tag="key2")
                    nc.scalar.activation(out=key2[:], in_=ps2[:], func=AF.Identity,
                                         bias=uneg_sb[:, b_, c:c + 1])
                    mxb = small.tile([128, 8], BF16, tag="mxb")
                    ixb = small.tile([128, 8], U32, tag="ixb")
                    nc.vector.max(out=mxb[:], in_=key2[:])
                    nc.vector.max_index(out=ixb[:], in_max=mxb[:], in_values=key2[:])
                    nc.gpsimd.indirect_dma_start(
                        out=npred[:, c, :], out_offset=None,
                        in_=ptabs[b_][:],
                        in_offset=IndirectOffsetOnAxis(ap=ixb[:, 0:1], axis=0))

                md = small.tile([128, NCH, 2], F32, tag="md")
                nc.vector.tensor_tensor(out=md[:], in0=npred[:], in1=gt_b[:],
                                        op=ALU.subtract)
                sabs = small.tile([128, NCH], F32, tag="sabs")
                nc.vector.tensor_reduce(out=sabs[:], in_=md[:], axis=AX.X,
                                        op=ALU.add, apply_absolute_value=True)
                smask = small.tile([128, NCH], F32, tag="smask")
                nc.vector.tensor_tensor(out=smask[:], in0=sabs[:], in1=mask_b[:],
                                        op=ALU.mult)
                nc.vector.tensor_reduce(out=res[:, 4 + b_:5 + b_], in_=smask[:],
                                        axis=AX.X, op=ALU.add)
                nc.vector.tensor_reduce(out=res[:, 8 + b_:9 + b_], in_=mask_b[:],
                                        axis=AX.X, op=ALU.add)

            nc.sync.dma_start(out=out[:], in_=res[:])

    nc.compile()
    return nc


_NC_CACHE = None


def _get_nc():
    global _NC_CACHE
    if _NC_CACHE is None:
        _NC_CACHE = build_nc()
    return _NC_CACHE


def make_in_maps(ini_pred_poly, pred_polys_, gt_polys, keyPointsMask):
    in_maps = []
    for i in range(NCORES):
        s = slice(BLOC * i, BLOC * (i + 1))
        ini = np.ascontiguousarray(ini_pred_poly[s], dtype=np.float32)
        p2 = np.ascontiguousarray(pred_polys_[s], dtype=np.float32)
        gp = np.ascontiguousarray(gt_polys[s], dtype=np.float32)
        km = np.ascontiguousarray(keyPointsMask[s], dtype=np.float32)
        hp = host_prep(ini, gp)
        im = {
            "ini_pred_poly": ini,
            "pred_polys_": p2,
            "gt_polys": gp,
            "keyPointsMask": km,
            "lhsQ": hp["lhsQ"],
            "rhsQ": hp["rhsQ"],
            "lhsR": hp["lhsR"],
            "rhsR": hp["rhsR"],
            "inv10_b": hp["inv10_b"],
            "negal_b": hp["negal_b"],
            "gtl8": hp["gtl8"],
            "prhs8": hp["prhs8"],
            "uneg": hp["uneg"],
        }
        for b_ in range(BLOC):
            im[f"itabw{b_}"] = np.ascontiguousarray(hp["itabw"][b_])
            im[f"ptab{b_}"] = np.ascontiguousarray(p2[b_])
        in_maps.append(im)
    return in_maps


def combine_outputs(outs):
    """outs: list of [128, 12] per-core partial sums -> scalar loss (float32)."""
    acc = np.zeros(12, dtype=np.float64)
    for o in outs:
        acc += o.astype(np.float64).sum(axis=0)
    s_p2g = acc[0:4].sum()          # sum |pred_polys_ - nearest_gt|
    s_g2p = acc[4:8].sum()          # sum mask * |nearest_pred - gt|
    s_msk = 2.0 * acc[8:12].sum()   # sum of broadcast mask
    loss_pred2gt = s_p2g / (B * NP * 2)
    loss = (s_g2p / (s_msk + 1.0) + loss_pred2gt) / 2.0
    return np.float32(loss)


def kernel(ini_pred_poly, pred_polys_, gt_polys, keyPointsMask):
    nc = _get_nc()
    in_maps = make_in_maps(ini_pred_poly, pred_polys_, gt_polys, keyPointsMask)
    r = run_bass_kernel_spmd(nc, in_maps, list(range(NCORES)))
    return combine_outputs([r.results[i]["out"] for i in range(NCORES)])


if __name__ == "__main__":
    import reference

    inputs = {k: np.asarray(v) for k, v in reference.setup_inputs().items()}
    got = kernel(**inputs)
    print("kernel loss:", got)


# revision 19
# speedup vs baseline: 3.5113x; 1.0835x over previous
"""Trainium2 Bass kernel for nn_DMLoss_61942018343083 (Chamfer-style polygon
matching loss, retrieval_knn).

Sharding: data-parallel over batch B=32 across 8 NeuronCores (4 batches/core).
Each core computes partial sums into a [128, 12] output tile; the host combines
them into the scalar loss.

v2 design (vs the fp32-matmul v1):

pred2gt (argmin over 5120 interp points for each of 512 preds):
  * Ranking key v[p, (t,i)] = -d^2(p, interp(t,i)) + |p-256|^2 computed on the
    PE as a K=14 bf16 matmul per (pred-chunk, t): coordinates are recentered by
    -256 and split hi/lo into bf16 pairs (p ~ p_hi + p_lo), so each product
    p*r = p_hi*r_hi + p_hi*r_lo + p_lo*r_hi is exact to ~1 unit (lo*lo
    dropped).  bf16 matmuls run at 1 cycle/column vs fp32's 4.
  * All 14-row operand blocks are HOST-PREPARED (numpy) and DMA'd in; the
    3-per-tile packing at base partitions 0/32/64 satisfies the PE constraint
    that lhsT/rhs share a base partition in {0,32,64}.
  * t=0..5 accumulate in a 6-bank PSUM tile, reduced with one vector
    tensor_reduce(max) over a strided [128, 512, 6] view; t=6..9 drain via
    scalar ACTIVATE(Identity, bias=-|p|^2) to self-scaled bf16 and merge with
    3 vector TT(max) ops.  The [128,5120] key is never materialized.
  * MAX8/FIND_INDEX8 on the final [128,512] column-max give the best segment
    i* per pred; ONE indirect DMA per chunk gathers that segment's 10 interp
    points (host-prepped i-major table [512, 10*2]); exact fp32 refine over
    the 10 candidates picks the true nearest (CPU-sim: rel err 1.4e-4).

gt2pred (argmin over 512 preds for each of 512 gts):
  * Same trick, orientation flipped: K=8 bf16 matmul per gt-chunk
    (psum = 2*g.p - |p|^2), scalar drain with bias -|g|^2 -> bf16 -d^2 key,
    top-1 via MAX8/FIND_INDEX8, gather pred row, masked abs-diff partials.
"""

import os
import sys

for _p in ("/opt/trn_rl_repo", "/root/.axon_site/_ro/trn_rl_repo"):
    if os.path.isdir(_p) and _p not in sys.path:
        sys.path.insert(0, _p)

import numpy as np
import ml_dtypes

bfloat16 = ml_dtypes.bfloat16

import concourse.bass as bass
import concourse.bacc as bacc
import concourse.mybir as mybir
from concourse.bass import IndirectOffsetOnAxis
from concourse.bass_utils import run_bass_kernel_spmd
from concourse.tile import TileContext

F32 = mybir.dt.float32
BF16 = mybir.dt.bfloat16
U32 = mybir.dt.uint32
AF = mybir.ActivationFunctionType
ALU = mybir.AluOpType
AX = mybir.AxisListType

B, NP, NG, T = 32, 512, 512, 10
NCORES = 8
BLOC = B // NCORES          # 4 batches per core
NCH = NP // 128             # 4 chunks of 128 preds / 128 gts
CEN = np.float32(256.0)     # recentering shift
NRED = 6                    # t-banks reduced on vector; T-NRED drained on scalar
K14 = 14                    # pred2gt contraction rows
K8 = 8                      # gt2pred contraction rows


def _split_hi_lo(x):
    x = np.asarray(x, dtype=np.float32)
    hi = x.astype(bfloat16)
    lo = (x - hi.astype(np.float32)).astype(bfloat16)
    return hi, lo


def host_prep(ini_pred_poly, gt_polys):
    """Build all matmul operands / tables for one core's BLOC batches."""
    f = np.float32
    a = (np.arange(T, dtype=np.float32) / f(T)).astype(np.float32)   # t/10
    b_ = (f(1.0) - a).astype(np.float32)

    ini = np.asarray(ini_pred_poly, dtype=np.float32)   # [BLOC, NP, 2]
    gt = np.asarray(gt_polys, dtype=np.float32)         # [BLOC, NG, 2]
    gtr = np.roll(gt, 1, axis=1)

    pc = ini - CEN
    gc = gt - CEN
    gcr = np.roll(gc, 1, axis=1)

    pxh, pxl = _split_hi_lo(pc[:, :, 0])
    pyh, pyl = _split_hi_lo(pc[:, :, 1])
    m1 = np.full_like(pxh, -1.0)

    # ---- pred2gt projection operands ----
    # Q[p,i] = (p - g_{i-1}) . s_i    (s = g_i - g_{i-1})
    # R[p,i] = -|p - g_{i-1}|^2  =  2 p.g_ - |g_|^2 - |p|^2   (centered coords)
    pp = (pc * pc).sum(-1).astype(np.float32)               # |p-256|^2 [BLOC, NP]
    s = (gc - gcr).astype(np.float32)
    len2 = (s * s).sum(-1).astype(np.float32)
    inv10 = np.where(len2 > 1e-6, (f(10.0) / len2).astype(np.float32),
                     np.float32(0.0)).astype(np.float32)
    negalpha = (-(len2 / f(100.0))).astype(np.float32)
    gs_ = (gcr * s).sum(-1).astype(np.float32)
    ug_ = (gcr * gcr).sum(-1).astype(np.float32)

    one = np.ones_like(pxh, dtype=np.float32)
    sxh, sxl = _split_hi_lo(s[:, :, 0])
    syh, syl = _split_hi_lo(s[:, :, 1])
    gsh, gsl = _split_hi_lo(gs_)
    lhsQ = np.stack([pxh, pxh, pxl, pyh, pyh, pyl, m1, m1],
                    axis=1).astype(bfloat16)                  # [BLOC, 8, NP]
    rhsQ = np.stack([sxh, sxl, sxh, syh, syl, syh, gsh, gsl],
                    axis=1).astype(bfloat16)                  # [BLOC, 8, NG]

    g2rxh, g2rxl = _split_hi_lo(f(2.0) * gcr[:, :, 0])
    g2ryh, g2ryl = _split_hi_lo(f(2.0) * gcr[:, :, 1])
    ugh, ugl = _split_hi_lo(ug_)
    npph, nppl = _split_hi_lo(-pp)
    oneb = one.astype(bfloat16)
    lhsR = np.stack([pxh, pxh, pxl, pyh, pyh, pyl, m1, m1, npph, nppl],
                    axis=1).astype(bfloat16)                  # [BLOC, 10, NP]
    rhsR = np.stack([g2rxh, g2rxl, g2rxh, g2ryh, g2ryl, g2ryh, ugh, ugl,
                     oneb, oneb], axis=1).astype(bfloat16)    # [BLOC, 10, NG]
    # per-column broadcast tiles (replicated across 128 partitions host-side)
    inv10_b = np.broadcast_to(inv10[:, None, :], (BLOC, 128, NG)).astype(bfloat16)
    negal_b = np.broadcast_to(negalpha[:, None, :], (BLOC, 128, NG)).copy()

    # ---- interp table, i-major wide rows [BLOC, NG, T*2] fp32 (bit-exact ref math)
    itabw = np.empty((BLOC, NG, T, 2), dtype=np.float32)
    for t in range(T):
        itabw[:, :, t, :] = (gt * a[t]).astype(np.float32) + (gtr * (f(1.0) - a[t])).astype(np.float32)
    itabw = itabw.reshape(BLOC, NG, T * 2)

    # ---- gt2pred: lhs rows [g2xh,g2xh,g2xl,g2yh,g2yh,g2yl,m1,m1] over gts
    g2xh, g2xl = _split_hi_lo(f(2.0) * gc[:, :, 0])
    g2yh, g2yl = _split_hi_lo(f(2.0) * gc[:, :, 1])
    m1g = np.full_like(g2xh, -1.0)
    gtl8 = np.stack([g2xh, g2xh, g2xl, g2yh, g2yh, g2yl, m1g, m1g],
                    axis=1).astype(bfloat16)                 # [BLOC, 8, NG]
    pph, ppl = _split_hi_lo(pp)
    prhs8 = np.stack([pxh, pxl, pxh, pyh, pyl, pyh, pph, ppl],
                     axis=1).astype(bfloat16)                # [BLOC, 8, NP]
    ug = (gc * gc).sum(-1).astype(np.float32)                # |g-256|^2 [BLOC, NG]
    uneg = (-ug).reshape(BLOC, NCH, 128).transpose(0, 2, 1).copy()

    return dict(lhsQ=lhsQ, rhsQ=rhsQ, lhsR=lhsR, rhsR=rhsR,
                inv10_b=inv10_b, negal_b=negal_b,
                itabw=itabw, gtl8=gtl8, prhs8=prhs8, uneg=uneg)


def build_nc():
    nc = bacc.Bacc()

    ini = nc.dram_tensor("ini_pred_poly", [BLOC, NP, 2], F32, kind="ExternalInput")
    pred2 = nc.dram_tensor("pred_polys_", [BLOC, NP, 2], F32, kind="ExternalInput")
    gt = nc.dram_tensor("gt_polys", [BLOC, NG, 2], F32, kind="ExternalInput")
    kmask = nc.dram_tensor("keyPointsMask", [BLOC, NG], F32, kind="ExternalInput")
    lhsQ_d = nc.dram_tensor("lhsQ", [BLOC, K8, NP], BF16, kind="ExternalInput")
    rhsQ_d = nc.dram_tensor("rhsQ", [BLOC, K8, NG], BF16, kind="ExternalInput")
    lhsR_d = nc.dram_tensor("lhsR", [BLOC, 10, NP], BF16, kind="ExternalInput")
    rhsR_d = nc.dram_tensor("rhsR", [BLOC, 10, NG], BF16, kind="ExternalInput")
    inv10_d = nc.dram_tensor("inv10_b", [BLOC, 128, NG], BF16, kind="ExternalInput")
    negal_d = nc.dram_tensor("negal_b", [BLOC, 128, NG], F32, kind="ExternalInput")
    gtl8_d = nc.dram_tensor("gtl8", [BLOC, K8, NG], BF16, kind="ExternalInput")
    prhs8_d = nc.dram_tensor("prhs8", [BLOC, K8, NP], BF16, kind="ExternalInput")
    uneg_d = nc.dram_tensor("uneg", [BLOC, 128, NCH], F32, kind="ExternalInput")
    # per-batch gather tables (offset-0 requirement for indirect DMA)
    itabws = [nc.dram_tensor(f"itabw{b_}", [NG, T * 2], F32, kind="ExternalInput")
              for b_ in range(BLOC)]
    ptabs = [nc.dram_tensor(f"ptab{b_}", [NP, 2], F32, kind="ExternalInput")
             for b_ in range(BLOC)]
    out = nc.dram_tensor("out", [128, 12], F32, kind="ExternalOutput")

    with TileContext(nc) as tc:
        with (
            tc.tile_pool(name="const", bufs=1) as cpool,
            tc.tile_pool(name="bat", bufs=2) as bat,
            tc.tile_pool(name="drain", bufs=3) as drp,
            tc.tile_pool(name="mrg", bufs=2) as mrg,
            tc.tile_pool(name="small", bufs=2) as small,
            tc.tile_pool(name="psA", bufs=2, space="PSUM") as psap,
            tc.tile_pool(name="psT", bufs=2, space="PSUM") as pstp,
        ):
            res = cpool.tile([128, 12], F32)
            nc.vector.memset(res[:], 0.0)
            c9 = cpool.tile([128, 1], F32)
            nc.vector.memset(c9[:], 9.0)
            candC = cpool.tile([128, BLOC, NCH, T, 2], F32)
            npredC = cpool.tile([128, BLOC, NCH, 2], F32)
            pxyC = cpool.tile([128, BLOC, NCH, 2], F32)
            pred2C = cpool.tile([128, BLOC, NCH, 2], F32)
            gtC = cpool.tile([128, BLOC, NCH, 2], F32)
            maskC = cpool.tile([128, BLOC, NCH], F32)
            uneg_sb = cpool.tile([128, BLOC, NCH], F32)
            nc.sync.dma_start(out=uneg_sb[:],
                              in_=uneg_d[:].rearrange("b p c -> p b c"))

            for b_ in range(BLOC):
                # ---------------- batch-level loads ----------------
                lhsQ = bat.tile([K8, NP], BF16, tag="lhsQ")
                nc.sync.dma_start(out=lhsQ[:], in_=lhsQ_d[b_])
                rhsQ = bat.tile([K8, NG], BF16, tag="rhsQ")
                nc.sync.dma_start(out=rhsQ[:], in_=rhsQ_d[b_])
                lhsR = bat.tile([10, NP], BF16, tag="lhsR")
                nc.scalar.dma_start(out=lhsR[:], in_=lhsR_d[b_])
                rhsR = bat.tile([10, NG], BF16, tag="rhsR")
                nc.scalar.dma_start(out=rhsR[:], in_=rhsR_d[b_])
                inv10_b = bat.tile([128, NG], BF16, tag="inv10_b")
                nc.scalar.dma_start(out=inv10_b[:], in_=inv10_d[b_])
                negal_b = bat.tile([128, NG], F32, tag="negal_b")
                nc.scalar.dma_start(out=negal_b[:], in_=negal_d[b_])
                gtl8 = bat.tile([K8, NG], BF16, tag="gtl8")
                nc.scalar.dma_start(out=gtl8[:], in_=gtl8_d[b_])
                prhs8 = bat.tile([K8, NP], BF16, tag="prhs8")
                nc.scalar.dma_start(out=prhs8[:], in_=prhs8_d[b_])
                nc.sync.dma_start(
                    out=pxyC[:, b_],
                    in_=ini[b_][:].rearrange("(m p) c -> p m c", m=NCH))
                nc.sync.dma_start(
                    out=pred2C[:, b_],
                    in_=pred2[b_][:].rearrange("(m p) c -> p m c", m=NCH))
                nc.sync.dma_start(
                    out=gtC[:, b_], in_=gt[b_][:].rearrange("(m p) c -> p m c", m=NCH))
                nc.sync.dma_start(
                    out=maskC[:, b_],
                    in_=kmask[b_][:].rearrange("(c p) -> p c", p=128))

                # ---------------- pred2gt (point-to-segment projection) ------
                # Q = w.s in PSUM, R = -|w|^2 in PSUM (w = p - g_{i-1}).
                # t* = clamp(round(10 Q / len2), 0, 9) via Relu-chain on scalar
                # vkey = R + t*(0.2 Q - (len2/100) t*) = -d^2(t*)  (fp32)
                for m in range(NCH):
                    sl = slice(128 * m, 128 * (m + 1))
                    psQ = psap.tile([128, NG], F32, tag="psQ")
                    nc.tensor.matmul(psQ[:], lhsT=lhsQ[:, sl], rhs=rhsQ[:],
                                     start=True, stop=True)
                    psR = psap.tile([128, NG], F32, tag="psR")
                    nc.tensor.matmul(psR[:], lhsT=lhsR[:, sl], rhs=rhsR[:],
                                     start=True, stop=True)
                    z = mrg.tile([128, NG], BF16, tag="z")
                    nc.vector.tensor_tensor(out=z[:], in0=psQ[:], in1=inv10_b[:],
                                            op=ALU.mult)
                    # clamp + round-to-nearest entirely on the scalar engine:
                    # r1 = max(z,0); r2 = max(9-r1,0); ym = -r2 + (9+2^23);
                    # ts = ym - 2^23  ->  clamp(round(z),0,9)
                    r1 = drp.tile([128, NG], F32, tag="r1")
                    nc.scalar.activation(out=r1[:], in_=z[:], func=AF.Relu)
                    r2 = drp.tile([128, NG], F32, tag="r2")
                    nc.scalar.activation(out=r2[:], in_=r1[:], func=AF.Relu,
                                         bias=c9[:, 0:1], scale=-1.0)
                    ym = drp.tile([128, NG], F32, tag="ym")
                    nc.scalar.activation(out=ym[:], in_=r2[:], func=AF.Copy,
                                         bias=8388617.0, scale=-1.0)
                    ts_ = drp.tile([128, NG], BF16, tag="ts_")
                    nc.scalar.activation(out=ts_[:], in_=ym[:], func=AF.Copy,
                                         bias=-8388608.0)
                    w1 = mrg.tile([128, NG], F32, tag="w1")
                    nc.vector.tensor_tensor(out=w1[:], in0=negal_b[:], in1=ts_[:],
                                            op=ALU.mult)
                    w2 = mrg.tile([128, NG], F32, tag="w2")
                    nc.vector.scalar_tensor_tensor(out=w2[:], in0=psQ[:], scalar=0.2,
                                                   in1=w1[:], op0=ALU.mult,
                                                   op1=ALU.add)
                    w3 = mrg.tile([128, NG], F32, tag="w3")
                    nc.vector.tensor_tensor(out=w3[:], in0=w2[:], in1=ts_[:],
                                            op=ALU.mult)
                    vkey = mrg.tile([128, NG], F32, tag="vkey")
                    nc.vector.scalar_tensor_tensor(out=vkey[:], in0=psR[:], scalar=1.0,
                                                   in1=w3[:], op0=ALU.mult,
                                                   op1=ALU.add)
                    mx8 = small.tile([128, 8], F32, tag="mx8")
                    i8 = small.tile([128, 8], U32, tag="i8")
                    nc.vector.max(out=mx8[:], in_=vkey[:])
                    nc.vector.max_index(out=i8[:], in_max=mx8[:], in_values=vkey[:])
                    nc.gpsimd.indirect_dma_start(
                        out=candC[:, b_, m].rearrange("p t c -> p (t c)"),
                        out_offset=None, in_=itabws[b_][:],
                        in_offset=IndirectOffsetOnAxis(ap=i8[:, 0:1], axis=0))

                # ---------------- gt2pred ----------------
                for c in range(NCH):
                    sl = slice(128 * c, 128 * (c + 1))
                    ps2 = pstp.tile([128, NP], F32, tag="psT")
                    nc.tensor.matmul(ps2[:], lhsT=gtl8[:, sl], rhs=prhs8[:],
                                     start=True, stop=True)
                    key2 = drp.tile([128, NP], BF16, tag="key2")
                    nc.scalar.activation(out=key2[:], in_=ps2[:], func=AF.Identity,
                                         bias=uneg_sb[:, b_, c:c + 1])
                    mxb = small.tile([128, 8], BF16, tag="mxb")
                    ixb = small.tile([128, 8], U32, tag="ixb")
                    nc.vector.max(out=mxb[:], in_=key2[:])
                    nc.vector.max_index(out=ixb[:], in_max=mxb[:], in_values=key2[:])
                    nc.gpsimd.indirect_dma_start(
                        out=npredC[:, b_, c, :], out_offset=None,
                        in_=ptabs[b_][:],
                        in_offset=IndirectOffsetOnAxis(ap=ixb[:, 0:1], axis=0))

            # ---------------- core-level refine + loss tails ----------------
            SH4 = [128, BLOC, NCH, T]
            dx = small.tile([128, BLOC, NCH, T], F32, tag="dx")
            dy = small.tile([128, BLOC, NCH, T], F32, tag="dy")
            nc.vector.tensor_tensor(
                out=dx[:], in0=candC[:, :, :, :, 0],
                in1=pxyC[:, :, :, 0:1].to_broadcast(SH4), op=ALU.subtract)
            nc.vector.tensor_tensor(
                out=dy[:], in0=candC[:, :, :, :, 1],
                in1=pxyC[:, :, :, 1:2].to_broadcast(SH4), op=ALU.subtract)
            sqx = small.tile([128, BLOC, NCH, T], F32, tag="sqx")
            sqy = small.tile([128, BLOC, NCH, T], F32, tag="sqy")
            dall = small.tile([128, BLOC, NCH, T], F32, tag="dall")
            nc.vector.tensor_tensor(out=sqx[:], in0=dx[:], in1=dx[:], op=ALU.mult)
            nc.vector.tensor_tensor(out=sqy[:], in0=dy[:], in1=dy[:], op=ALU.mult)
            nc.vector.tensor_tensor(out=dall[:], in0=sqx[:], in1=sqy[:], op=ALU.add)
            dmin = small.tile([128, BLOC, NCH], F32, tag="dmin")
            nc.vector.tensor_reduce(out=dmin[:], in_=dall[:], axis=AX.X, op=ALU.min)
            sel = small.tile([128, BLOC, NCH, T], F32, tag="sel")
            nc.vector.tensor_tensor(
                out=sel[:], in0=dall[:],
                in1=dmin[:].unsqueeze(3).to_broadcast(SH4), op=ALU.is_equal)
            selx = small.tile([128, BLOC, NCH, T], F32, tag="selx")
            sely = small.tile([128, BLOC, NCH, T], F32, tag="sely")
            nc.vector.tensor_tensor(out=selx[:], in0=sel[:],
                                    in1=candC[:, :, :, :, 0], op=ALU.mult)
            nc.vector.tensor_tensor(out=sely[:], in0=sel[:],
                                    in1=candC[:, :, :, :, 1], op=ALU.mult)
            nx = small.tile([128, BLOC, NCH], F32, tag="nx")
            ny = small.tile([128, BLOC, NCH], F32, tag="ny")
            cnt = small.tile([128, BLOC, NCH], F32, tag="cnt")
            nc.vector.tensor_reduce(out=nx[:], in_=selx[:], axis=AX.X, op=ALU.add)
            nc.vector.tensor_reduce(out=ny[:], in_=sely[:], axis=AX.X, op=ALU.add)
            nc.vector.tensor_reduce(out=cnt[:], in_=sel[:], axis=AX.X, op=ALU.add)
            rcnt = small.tile([128, BLOC, NCH], F32, tag="rcnt")
            nc.vector.reciprocal(out=rcnt[:], in_=cnt[:])
            df = small.tile([128, BLOC, NCH, 2], F32, tag="df")
            nxn = small.tile([128, BLOC, NCH], F32, tag="nxn")
            nyn = small.tile([128, BLOC, NCH], F32, tag="nyn")
            nc.vector.tensor_tensor(out=nxn[:], in0=nx[:], in1=rcnt[:], op=ALU.mult)
            nc.vector.tensor_tensor(out=nyn[:], in0=ny[:], in1=rcnt[:], op=ALU.mult)
            nc.vector.tensor_tensor(out=df[:, :, :, 0], in0=pred2C[:, :, :, 0],
                                    in1=nxn[:], op=ALU.subtract)
            nc.vector.tensor_tensor(out=df[:, :, :, 1], in0=pred2C[:, :, :, 1],
                                    in1=nyn[:], op=ALU.subtract)
            nc.vector.tensor_reduce(out=res[:, 0:BLOC], in_=df[:], axis=AX.XY,
                                    op=ALU.add, apply_absolute_value=True)

            md = small.tile([128, BLOC, NCH, 2], F32, tag="md")
            nc.vector.tensor_tensor(out=md[:], in0=npredC[:], in1=gtC[:],
                                    op=ALU.subtract)
            sabs = small.tile([128, BLOC, NCH], F32, tag="sabs")
            nc.vector.tensor_reduce(out=sabs[:], in_=md[:], axis=AX.X,
                                    op=ALU.add, apply_absolute_value=True)
            smask = small.tile([128, BLOC, NCH], F32, tag="smask")
            nc.vector.tensor_tensor(out=smask[:], in0=sabs[:], in1=maskC[:],
                                    op=ALU.mult)
            nc.vector.tensor_reduce(out=res[:, 4:4 + BLOC], in_=smask[:],
                                    axis=AX.X, op=ALU.add)
            nc.vector.tensor_reduce(out=res[:, 8:8 + BLOC], in_=maskC[:],
                                    axis=AX.X, op=ALU.add)

            nc.sync.dma_start(out=out[:], in_=res[:])

    nc.compile()
    return nc


_NC_CACHE = None


def _get_nc():
    global _NC_CACHE
    if _NC_CACHE is None:
        _NC_CACHE = build_nc()
    return _NC_CACHE


def make_in_maps(ini_pred_poly, pred_polys_, gt_polys, keyPointsMask):
    in_maps = []
    for i in range(NCORES):
        s = slice(BLOC * i, BLOC * (i + 1))
        ini = np.ascontiguousarray(ini_pred_poly[s], dtype=np.float32)
        p2 = np.ascontiguousarray(pred_polys_[s], dtype=np.float32)
        gp = np.ascontiguousarray(gt_polys[s], dtype=np.float32)
        km = np.ascontiguousarray(keyPointsMask[s], dtype=np.float32)
        hp = host_prep(ini, gp)
        im = {
            "ini_pred_poly": ini,
            "pred_polys_": p2,
            "gt_polys": gp,
            "keyPointsMask": km,
            "lhsQ": hp["lhsQ"],
            "rhsQ": hp["rhsQ"],
            "lhsR": hp["lhsR"],
            "rhsR": hp["rhsR"],
            "inv10_b": hp["inv10_b"],
            "negal_b": hp["negal_b"],
            "gtl8": hp["gtl8"],
            "prhs8": hp["prhs8"],
            "uneg": hp["uneg"],
        }
        for b_ in range(BLOC):
            im[f"itabw{b_}"] = np.ascontiguousarray(hp["itabw"][b_])
            im[f"ptab{b_}"] = np.ascontiguousarray(p2[b_])
        in_maps.append(im)
    return in_maps


def combine_outputs(outs):
    """outs: list of [128, 12] per-core partial sums -> scalar loss (float32)."""
    acc = np.zeros(12, dtype=np.float64)
    for o in outs:
        acc += o.astype(np.float64).sum(axis=0)
    s_p2g = acc[0:4].sum()          # sum |pred_polys_ - nearest_gt|
    s_g2p = acc[4:8].sum()          # sum mask * |nearest_pred - gt|
    s_msk = 2.0 * acc[8:12].sum()   # sum of broadcast mask
    loss_pred2gt = s_p2g / (B * NP * 2)
    loss = (s_g2p / (s_msk + 1.0) + loss_pred2gt) / 2.0
    return np.float32(loss)


def kernel(ini_pred_poly, pred_polys_, gt_polys, keyPointsMask):
    nc = _get_nc()
    in_maps = make_in_maps(ini_pred_poly, pred_polys_, gt_polys, keyPointsMask)
    r = run_bass_kernel_spmd(nc, in_maps, list(range(NCORES)))
    return combine_outputs([r.results[i]["out"] for i in range(NCORES)])


if __name__ == "__main__":
    import reference

    inputs = {k: np.asarray(v) for k, v in reference.setup_inputs().items()}
    got = kernel(**inputs)
    print("kernel loss:", got)
